# revision 1
# baseline (speedup 1.0000x reference)
"""DRConv2d Trainium2 kernel — batch-parallel over 8 NeuronCores.

Per core (one sample b): x_b [64, 126, 126] -> out_b [64, 124, 124]

Math (per sample):
  pooled = avgpool3x3(x); s1 = sigmoid(w1 @ pooled + b1)
  kern[g] = w2[g] @ s1[g] + b2[g]          -> per-sample filter bank [8*64, 64, 3, 3]
  out_r = conv3x3_valid(x, kern)           -> [8, 64, 124, 124]
  guide = conv3x3_valid(x, wg) + bg        -> [8, 124, 124]
  out = out_{argmax_r guide}               -> [64, 124, 124]

Device strategy:
  - host re-encodes x as fp16 hi/lo pair (x = xh + xl + O(2^-22)) in two
    partition-stacked shifted layouts (xa = [v@0; v@+1], x2 = [v@0; v@+126]),
    so the 9-tap valid conv runs as 5 PSUM-accumulated matmuls per
    128-channel output chunk (tap pairing doubles contraction to K=128).
  - main conv in fp16 (error ~5e-4, well under tolerance).
  - guide conv compensated fp16x2: wh@xh + wh@xl + wl@xh (error ~1e-6) so
    its argmax matches the fp32 reference except at ~1e-6 margins.
  - region one-hot mask: gpsimd partition all-reduce max + DVE is_equal -> fp8.
  - selection: emask_c = E_c.T @ mask (fp8 matmul), t_c = psum_c * emask_c
    (DVE), out = sum_c ones.T @ t_c (accumulating fold matmuls).
  - one interleaved loop over 31 position tiles (4 output rows each), PSUM
    pools shared so banks rotate without phase barriers.
"""
import numpy as np

import concourse.bass as bass
import concourse.mybir as mybir
import concourse.tile as tile
from concourse import bacc, bass_isa, library_config
from concourse.bass_utils import run_bass_kernel_spmd

F32 = mybir.dt.float32
F16 = mybir.dt.float16
FP8 = mybir.dt.float8e4
AL = mybir.AluOpType
AFT = mybir.ActivationFunctionType

R, CIN, COUT = 8, 64, 64
H, W = 126, 126
HO, WO = 124, 124
NPOS = HO * WO          # 15376
HW2 = H * W             # 15876
NT = 31                 # position tiles, 4 output rows each
TN = 4 * WO             # 496 positions per tile
SUP = 2                 # sub-tiles per super-tile

# (src, dy, dx, hw-indices): pair windows on xa=[v@0;v@+1], x2=[v@0;v@+126]
PAIRS = [
    ("xa", 0, 0, (0, 1)),
    ("xa", 1, 0, (3, 4)),
    ("xa", 2, 0, (6, 7)),
    ("x2", 0, 2, (2, 5)),
    ("x2", 2, 2, (8,)),
]

XNAMES = ("xah", "xal", "x2h")


def _win(xv, t, dy, dx, k=128):
    r0 = 4 * t
    return xv[0:k, r0 + dy: r0 + dy + 4, dx: dx + WO]


def build_module():
    nc = bacc.Bacc(trn_type="TRN2", target_bir_lowering=False, debug=False,
                   num_devices=8)

    d_xs = {n: nc.dram_tensor(n, (128, HW2), F16, kind="ExternalInput")
            for n in XNAMES}
    d_w1aug = nc.dram_tensor("w1aug", (65, 64), F32, kind="ExternalInput")
    d_krhs = nc.dram_tensor("krhs", (72, 4096), F16, kind="ExternalInput")
    d_sdelta = nc.dram_tensor("sdelta", (8, 72), F16, kind="ExternalInput")
    d_wgph = nc.dram_tensor("wgph", (128, 8, 8), F16, kind="ExternalInput")
    d_wgpl = nc.dram_tensor("wgpl", (128, 5, 8), F16, kind="ExternalInput")
    d_bg = nc.dram_tensor("bg8", (8, 1), F32, kind="ExternalInput")
    d_ec = nc.dram_tensor("ec", (128, 4, 128), F32, kind="ExternalInput")
    d_ones = nc.dram_tensor("ones64", (128, 64), F16, kind="ExternalInput")
    d_y = nc.dram_tensor("y", (COUT, NPOS), F32, kind="ExternalOutput")

    with tile.TileContext(nc) as tc:
        kernel_body(nc, tc, d_xs, d_w1aug, d_krhs, d_sdelta, d_wgph,
                    d_wgpl, d_bg, d_ec, d_ones, d_y)
    nc.compile()
    return nc


def kernel_body(nc, tc, d_xs, d_w1aug, d_krhs, d_sdelta, d_wgph, d_wgpl,
                d_bg, d_ec, d_ones, d_y):
    nc.gpsimd.load_library(library_config.mlp)

    with (
        tc.tile_pool(name="sbx", bufs=1) as sbx,
        tc.tile_pool(name="sbw", bufs=1) as sbw,
        tc.tile_pool(name="sbk", bufs=1) as sbk,
        tc.tile_pool(name="sbm", bufs=1) as sbm,
        tc.tile_pool(name="sbg", bufs=3) as sbg,
        tc.tile_pool(name="sbt", bufs=8) as sbt,
        tc.tile_pool(name="sbe", bufs=3) as sbe,
        tc.tile_pool(name="sbo", bufs=3) as sbo,
        tc.tile_pool(name="p1", bufs=4, space="PSUM") as p1,   # kern, then chunks
        tc.tile_pool(name="p2", bufs=2, space="PSUM") as p2,   # guide / emask
        tc.tile_pool(name="p3", bufs=2, space="PSUM") as p3,   # s1, then fold
    ):
        # ---- small weights (guide-gating ones first on the ring) ----
        wgph = sbw.tile([128, 8, 8], F16, tag="wgph")
        nc.sync.dma_start(wgph[:], d_wgph.ap())
        wgpl = sbw.tile([128, 5, 8], F16, tag="wgpl")
        nc.sync.dma_start(wgpl[:], d_wgpl.ap())
        bg8 = sbw.tile([8, 1], F32, tag="bg8")
        nc.sync.dma_start(bg8[:], d_bg.ap())
        w1aug = sbw.tile([65, 64], F32, tag="w1aug")
        nc.sync.dma_start(w1aug[:], d_w1aug.ap())
        ecf = sbw.tile([128, 4, 128], F32, tag="ecf")
        nc.sync.dma_start(ecf[:], d_ec.ap())
        ec8 = sbw.tile([128, 4, 128], FP8, tag="ec8")
        nc.vector.tensor_copy(ec8[:], ecf[:])
        ones = sbw.tile([128, 64], F16, tag="ones")
        nc.sync.dma_start(ones[:], d_ones.ap())
        LW = sbw.tile([128, 20, 128], F16, tag="LW")
        krhs = sbk.tile([72, 4096], F16, tag="krhs")
        nc.sync.dma_start(krhs[:], d_krhs.ap())

        # ---- x: pre-shifted fp16 hi/lo layouts, quarter-granular DMAs ----
        xt = {}
        NQ = 8
        Q = (HW2 + NQ - 1) // NQ
        for n in XNAMES:
            xt[n] = sbx.tile([128, HW2], F16, tag=n, name=n)
        for qi in range(NQ):
            lo, hi = Q * qi, min(Q * (qi + 1), HW2)
            for n in XNAMES:
                dma = nc.sync if n.endswith("h") else nc.scalar
                dma.dma_start(xt[n][:, lo:hi], d_xs[n].ap()[:, lo:hi])
        xv = {n: xt[n][:].rearrange("p (h w) -> p h w", h=H) for n in XNAMES}

        # ---- interleaved main loop: guide+mask+conv+select per tile ----
        mask = sbm.tile([128, NPOS], FP8, tag="mask")

        wgpx = wgph

        def guide_tile(t):
            pg = p2.tile([128, TN], F32, tag="pg", name=f"pg{t}")
            steps = []
            for wsel in ("h", "l"):
                for p, (src, dy, dx, hws) in enumerate(PAIRS):
                    k = 128 if len(hws) == 2 else 64
                    xvw = xv[("xa" if src == "xa" else "x2") + "h"]
                    wt = wgph if wsel == "h" else wgpl
                    steps.append((wt, p, xvw, dy, dx, k))
            # correction pass wh@xl from xa_l only: paired taps for xa pairs,
            # single top-half taps for the x2 pairs (taps (0,2),(1,2),(2,2))
            for p, (src, dy, dx, hws) in enumerate(PAIRS):
                if src == "xa":
                    k = 128 if len(hws) == 2 else 64
                    steps.append((wgph, p, xv["xal"], dy, dx, k))
            for hw, (wcol, dy, dx) in ((2, (5, 0, 2)), (5, (6, 1, 2)), (8, (7, 2, 2))):
                steps.append((wgpx, wcol, xv["xal"], dy, dx, 64))
            # 4-way column tiling: strips j at psum partitions 32j..32j+8
            # accumulate 4 of the 16 terms each and run concurrently on PE
            for i, (wt, p, xvw, dy, dx, k) in enumerate(steps):
                j = i % 4
                nc.tensor.matmul(pg[32 * j:32 * j + 8, :], wt[0:k, p, :],
                                 _win(xvw, t, dy, dx, k),
                                 start=(i < 4), stop=(i >= len(steps) - 4),
                                 tile_position=(0, 32 * j),
                                 skip_group_check=True)
            gs0 = sbg.tile([8, TN], F32, tag="gs0", name=f"gs0_{t}")
            nc.scalar.copy(gs0[:], pg[0:8, :])
            u1 = sbg.tile([8, TN], F32, tag="u1", name=f"u1_{t}")
            nc.vector.tensor_tensor(u1[:], gs0[:], pg[32:40, :], op=AL.add)
            u2 = sbg.tile([8, TN], F32, tag="u2", name=f"u2_{t}")
            nc.vector.tensor_tensor(u2[:], u1[:], pg[64:72, :], op=AL.add)
            g_s = sbg.tile([8, TN], F32, tag="g_s", name=f"gs{t}")
            nc.vector.scalar_tensor_tensor(g_s[:], u2[:], bg8[:], pg[96:104, :],
                                           op0=AL.add, op1=AL.add)
            gm = sbg.tile([8, TN], F32, tag="gm", name=f"gm{t}")
            nc.gpsimd.partition_all_reduce(gm[:], g_s[:], channels=8,
                                           reduce_op=bass_isa.ReduceOp.max)
            jm = 32 * (t % 4)
            nc.vector.tensor_tensor(mask[jm:jm + 8, TN * t: TN * (t + 1)],
                                    g_s[:], gm[:], op=AL.is_equal)

        # pooled reduce first in DVE order; guide tiles 0..G-1 keep PE busy
        # while it completes (PE executes in program order per engine).
        rs = sbg.tile([64, 378], F16, tag="rs")
        xrow = xt["xah"][0:64, :].rearrange("p (a b) -> p a b", b=42)
        with nc.allow_low_precision(reason="pooled feeds sigmoid; fp16 rowsums are plenty"):
            for r0 in range(0, 378, 48):
                r1 = min(r0 + 48, 378)
                nc.vector.tensor_reduce(
                    rs[:, r0:r1], xrow[:, r0:r1, :],
                    axis=mybir.AxisListType.X, op=AL.add)
        # pooled columns are written in "slot" order: taps of each conv pair
        # sit in adjacent slots so kern psum rows come out pair-adjacent
        # (slot(hw): 0,1,3,4,6,7,2,5,8 -> 0..8)
        pooled = sbg.tile([65, 9], F32, tag="pooled")
        rsv = rs[:].rearrange("p (kh r kw) -> p kh kw r", kh=3, r=42, kw=3)
        for kh in range(3):
            nc.vector.tensor_reduce(
                pooled[0:64, 2 * kh: 2 * kh + 2], rsv[:, kh, 0:2, :],
                axis=mybir.AxisListType.X, op=AL.add)
            nc.vector.tensor_reduce(
                pooled[0:64, 6 + kh: 7 + kh], rsv[:, kh, 2:3, :],
                axis=mybir.AxisListType.X, op=AL.add)
        nc.vector.memset(pooled[64:65, :], 1.0)

        G = 12
        for t in range(G):
            guide_tile(t)

        # ---- generator network: s1 -> kern -> LW ----
        s1p = p3.tile([64, 9], F32, tag="pf")
        nc.tensor.matmul(s1p[:], w1aug[:], pooled[:], start=True, stop=True)
        s1s = sbg.tile([64, 9], F16, tag="s1s")
        nc.scalar.activation(s1s[:], s1p[:], AFT.Sigmoid)

        S = sbk.tile([72, 72], F16, tag="S")
        nc.vector.memset(S[:], 0.0)
        nc.sync.dma_start(S[64:72, :], d_sdelta.ap())
        for g in range(8):
            nc.sync.dma_start(S[8 * g:8 * g + 8, 9 * g:9 * g + 9],
                              s1s[8 * g:8 * g + 8, :])

        ksb = sbk.tile([128, 4096], F16, tag="ksb")
        for j in range(8):
            pk = p1.tile([72, 512], F32, tag="pc", name=f"pk{j}")
            nc.tensor.matmul(pk[:], S[:], krhs[:, 512 * j: 512 * (j + 1)],
                             start=True, stop=True)
            nc.scalar.copy(ksb[0:72, 512 * j: 512 * (j + 1)], pk[:])

        # kern -> conv lhsT tiles: LW[64*s + cin, c*5 + p, 64*h + cout]
        # one DMA per (c, pair, h): both taps fused (dst partitions are
        # (s,cin)-linear; src rows strided by hwB-hwA)
        ksv = ksb[0:72, :].rearrange("p (ci co) -> p ci co", ci=64)
        # kern rows are (g, slot) with pair taps adjacent: pair p -> slots
        # (2p, 2p+1) for p<4, slot 8 for the single tap
        ndma = 0
        for c in range(4):
            for p, (_, _, _, hws) in enumerate(PAIRS):
                for hh in range(2):
                    g = 2 * c + hh
                    ring = nc.sync if ndma % 2 else nc.scalar
                    ndma += 1
                    if len(hws) == 2:
                        ring.dma_start(
                            LW[0:128, c * 5 + p, 64 * hh:64 * hh + 64],
                            ksv[g * 9 + 2 * p: g * 9 + 2 * p + 2, :, :])
                    else:
                        ring.dma_start(
                            LW[0:64, c * 5 + p, 64 * hh:64 * hh + 64],
                            ksv[g * 9 + 8: g * 9 + 9, :, :])

        supers = [list(range(s, min(s + SUP, NT))) for s in range(0, NT, SUP)]
        for ts in supers:
            for t in ts:
                if t >= G:
                    guide_tile(t)
            tcts = {}
            for c in range(4):
                pcs = {t: p1.tile([128, TN], F32, tag="pc",
                                  name=f"pc_{c}_{t}") for t in ts}
                for p, (src, dy, dx, hws) in enumerate(PAIRS):
                    k = 128 if len(hws) == 2 else 64
                    xvw = xv[("xa" if src == "xa" else "x2") + "h"]
                    lw = LW[0:k, c * 5 + p, :]
                    for t in ts:
                        nc.tensor.matmul(
                            pcs[t][:], lw, _win(xvw, t, dy, dx, k),
                            start=(p == 0), stop=(p == len(PAIRS) - 1))
                for t in ts:
                    em = p2.tile([128, TN], F32, tag="pg", name=f"em_{c}_{t}")
                    jm = 32 * (t % 4)
                    nc.tensor.matmul(em[:], ec8[jm:jm + 8, c, :],
                                     mask[jm:jm + 8, TN * t: TN * (t + 1)],
                                     start=True, stop=True,
                                     tile_position=(jm, 0),
                                     skip_group_check=True)
                    em_s = sbe.tile([128, TN], F16, tag="em_s",
                                    name=f"ems_{c}_{t}")
                    nc.scalar.copy(em_s[:], em[:])
                    tct = sbt.tile([128, TN], F16, tag="tct",
                                   name=f"tct_{c}_{t}")
                    nc.vector.tensor_tensor(tct[:], pcs[t][:], em_s[:],
                                            op=AL.mult)
                    tcts[(c, t)] = tct
            # fold matmuls are M=64: col-tile the two sub-tiles' folds into
            # disjoint 64-column groups so they run concurrently on PE
            ost = sbo.tile([64, len(ts) * TN], F32, tag="ost",
                           name=f"ost{ts[0]}")
            pf = p3.tile([128, TN], F32, tag="pf", name=f"pf{ts[0]}")
            for c in range(4):
                for i, t in enumerate(ts):
                    nc.tensor.matmul(pf[64 * i:64 * i + 64, :], ones[:],
                                     tcts[(c, t)][:],
                                     start=(c == 0), stop=(c == 3),
                                     tile_position=(0, 64 * i),
                                     skip_group_check=True)
            for i, t in enumerate(ts):
                nc.scalar.copy(ost[:, TN * i: TN * (i + 1)],
                               pf[64 * i:64 * i + 64, :])
            nc.sync.dma_start(d_y.ap()[:, TN * ts[0]: TN * (ts[-1] + 1)], ost[:])


def host_prep(w1, b1, w2, b2, wg, bg):
    w1p = (np.asarray(w1, np.float32).T / 1764.0)
    w1aug = np.concatenate([w1p, np.asarray(b1, np.float32)[None, :]], 0)

    w2v = np.asarray(w2, np.float32).reshape(R, COUT, CIN, R)
    w2r = w2v.transpose(0, 3, 2, 1).reshape(64, CIN * COUT)
    b2v = np.asarray(b2, np.float32).reshape(R, COUT, CIN)
    b2r = b2v.transpose(0, 2, 1).reshape(R, CIN * COUT)
    krhs = np.ascontiguousarray(
        np.concatenate([w2r, b2r], 0).astype(np.float16))

    sdelta = np.zeros((8, 72), np.float16)
    for g in range(8):
        sdelta[g, 9 * g: 9 * g + 9] = 1.0

    wgv = np.asarray(wg, np.float32)
    wgp = np.zeros((128, 8, 8), np.float32)
    for p, (_, _, _, hws) in enumerate(PAIRS):
        for s, hw in enumerate(hws):
            dy, dx = hw // 3, hw % 3
            wgp[64 * s:64 * s + 64, p, :] = wgv[:, :, dy, dx].T
    for col, hw in ((5, 2), (6, 5), (7, 8)):
        dy, dx = hw // 3, hw % 3
        wgp[0:64, col, :] = wgv[:, :, dy, dx].T
    wgph = wgp.astype(np.float16)
    wgpl = (wgp[:, 0:5, :] - wgph[:, 0:5, :].astype(np.float32)).astype(np.float16)

    bg8 = np.asarray(bg, np.float32).reshape(8, 1)

    ec = np.zeros((128, 4, 128), np.float32)
    for j in range(4):
        for c in range(4):
            ec[32 * j + 2 * c, c, 0:64] = 1.0
            ec[32 * j + 2 * c + 1, c, 64:128] = 1.0

    ones64 = np.zeros((128, 64), np.float16)
    for k in range(128):
        ones64[k, k % 64] = 1.0

    return dict(w1aug=np.ascontiguousarray(w1aug), krhs=krhs, sdelta=sdelta,
                wgph=wgph, wgpl=wgpl, bg8=bg8, ec=ec, ones64=ones64)


def shard_x(xb):
    """One sample [64, 126, 126] -> 4 pre-shifted fp16 hi/lo SBUF layouts."""
    xf = np.ascontiguousarray(np.asarray(xb, np.float32).reshape(CIN, HW2))
    xh = xf.astype(np.float16)
    xl = (xf - xh.astype(np.float32)).astype(np.float16)
    out = {}
    for n, base, shift in (("xah", xh, 1), ("xal", xl, 1),
                           ("x2h", xh, 126)):
        t = np.zeros((128, HW2), np.float16)
        t[0:64] = base
        t[64:128, 0:HW2 - shift] = base[:, shift:]
        out[n] = t
    return out


_NC_CACHE = {}


def kernel(x, w1, b1, w2, b2, wg, bg, _profile=None):
    x = np.asarray(x, np.float32)
    Bn = x.shape[0]
    assert Bn == 8
    weights = host_prep(w1, b1, w2, b2, wg, bg)

    if "nc" not in _NC_CACHE:
        _NC_CACHE["nc"] = build_module()
    nc = _NC_CACHE["nc"]

    in_maps = []
    for b in range(Bn):
        m = shard_x(x[b])
        m.update(weights)
        in_maps.append(m)

    kwargs = dict(_profile.get("kwargs", {})) if _profile else {}
    res = run_bass_kernel_spmd(nc, in_maps, core_ids=list(range(Bn)), **kwargs)
    if _profile is not None:
        _profile["result"] = res

    out = np.stack([res.results[b]["y"].reshape(COUT, HO, WO) for b in range(Bn)])
    return out.astype(np.float32)



# revision 10
# speedup vs baseline: 1.3973x; 1.3973x over previous
"""DRConv2d Trainium2 kernel — batch-parallel over 8 NeuronCores.

Per core (one sample b): x_b [64, 126, 126] -> out_b [64, 124, 124]

Math (per sample):
  pooled = avgpool3x3(x); s1 = sigmoid(w1 @ pooled + b1)
  kern[g] = w2[g] @ s1[g] + b2[g]          -> per-sample filter bank [8*64, 64, 3, 3]
  out_r = conv3x3_valid(x, kern)           -> [8, 64, 124, 124]
  guide = conv3x3_valid(x, wg) + bg        -> [8, 124, 124]
  out = out_{argmax_r guide}               -> [64, 124, 124]

Device strategy (cost-model-aware: matmul cost = out_free_size x dtype_rate,
independent of K and M; fp8 DoubleRow = 0.5x):
  - main conv fp16: x as two partition-stacked shifted layouts
    xa16=[v@0;v@+1], x216=[v@0;v@+126]; 9-tap conv = 4 paired K=128 matmuls
    per 128-channel chunk + 1 combined matmul (see below).
  - guide conv: fp16 main pass (5 matmuls) + compensated corrections
    (wl@x + wh@xl) in fp8 DoubleRow (6 matmuls at 0.5 = 3 slots), keeping
    argmax flips at the fp16x2-compensated level.
  - region one-hot mask: gpsimd partition all-reduce max + DVE is_equal.
  - selection: BIG*mask injected into each conv psum chunk through the
    combined matmul (tap-8 weights rows 0:64 + BIG*E_c rows 64:72, rhs is a
    staged [72,TN] tile holding the tap-8 x window + mask rows), then
    u_c = relu(P_c - BIG/2) and a DVE add-tree; no emask/fold matmuls.
  - one interleaved loop over 31 position tiles (4 output rows each).
"""
import numpy as np
import ml_dtypes

import concourse.bass as bass
import concourse.mybir as mybir
import concourse.tile as tile
from concourse import bacc, bass_isa, library_config
from concourse.bass_utils import run_bass_kernel_spmd

F32 = mybir.dt.float32
F16 = mybir.dt.float16
FP8 = mybir.dt.float8e4
FP8E5 = mybir.dt.float8e5
AL = mybir.AluOpType
AFT = mybir.ActivationFunctionType
DR = mybir.MatmulPerfMode.DoubleRow

R, CIN, COUT = 8, 64, 64
H, W = 126, 126
HO, WO = 124, 124
NPOS = HO * WO          # 15376
HW2 = H * W             # 15876
NT = 31                 # position tiles, 4 output rows each
TN = 4 * WO             # 496 positions per tile
G = 14                  # guide tiles run ahead of conv
GPRE = 6                # guide tiles before the generator block
BIG = 64.0              # mask inject magnitude; relu offset is BIG/2
CSC = 2.0 ** -8         # guide correction scale (host pre-scales by 2^8)

# conv/guide tap pairing: kern rows are (g, slot), pair taps at adjacent
# slots: slot -> hw: (0,1),(3,4),(6,7),(2,5) pairs + tap8 at slot 8
SLOT_HW = (0, 1, 3, 4, 6, 7, 2, 5, 8)
CPAIRS = [("xa16", 0, 0), ("xa16", 1, 0), ("xa16", 2, 0), ("x216", 0, 2)]
GMAIN = [("xa16", 0, 0, 128), ("xa16", 1, 0, 128), ("xa16", 2, 0, 128),
         ("x216", 0, 2, 128), ("x216", 2, 2, 64)]
# guide corr DR rhs windows: (ktile_stride, dy, dx)
CORR_SPECS = [(W, 0, 0), (2, 2, 0), (W, 0, 2)]


def _win(xv, t, dy, dx, k=128):
    r0 = 4 * t
    return xv[0:k, r0 + dy: r0 + dy + 4, dx: dx + WO]


def _corr_ap(xt, t, ktile_stride, dy, dx):
    """Manual DR rhs AP: [partition 128][ktile 2][row 4][col 124] with
    overlapping strides on the flat [128, HW2] x tile."""
    base = xt[:]
    off = base.offset + (4 * t + dy) * W + dx
    return bass.AP(base.tensor, off, [[HW2, 128], [ktile_stride, 2],
                                      [W, 4], [1, WO]])


def build_module():
    nc = bacc.Bacc(trn_type="TRN2", target_bir_lowering=False, debug=False,
                   num_devices=8)

    d_xs = {}
    for n, dt in (("xa16", F16), ("x216", F16), ("xa8", FP8), ("gxa8", FP8E5)):
        d_xs[n] = nc.dram_tensor(n, (128, HW2), dt, kind="ExternalInput")
    d_w1aug = nc.dram_tensor("w1aug", (65, 64), F32, kind="ExternalInput")
    d_krhs = nc.dram_tensor("krhs", (72, 4096), F16, kind="ExternalInput")
    d_sdelta = nc.dram_tensor("sdelta", (8, 72), F16, kind="ExternalInput")
    d_wgp16 = nc.dram_tensor("wgp16", (128, 5, 8), F16, kind="ExternalInput")
    d_wl = nc.dram_tensor("wl8", (128, 3, 2, 16), FP8E5, kind="ExternalInput")
    d_wh = nc.dram_tensor("wh8", (128, 3, 2, 16), FP8, kind="ExternalInput")
    d_bg = nc.dram_tensor("bg8", (8, 1), F32, kind="ExternalInput")
    d_ec = nc.dram_tensor("ec16", (8, 4, 128), F16, kind="ExternalInput")
    d_ones = nc.dram_tensor("ones64", (128, 64), F16, kind="ExternalInput")
    d_y = nc.dram_tensor("y", (COUT, NPOS), F32, kind="ExternalOutput")

    with tile.TileContext(nc) as tc:
        kernel_body(nc, tc, d_xs, d_w1aug, d_krhs, d_sdelta, d_wgp16,
                    d_wl, d_wh, d_bg, d_ec, d_ones, d_y)
    nc.compile()
    return nc


def kernel_body(nc, tc, d_xs, d_w1aug, d_krhs, d_sdelta, d_wgp16,
                d_wl, d_wh, d_bg, d_ec, d_ones, d_y):
    nc.gpsimd.load_library(library_config.mlp)

    with (
        tc.tile_pool(name="sbx", bufs=1) as sbx,
        tc.tile_pool(name="sbw", bufs=1) as sbw,
        tc.tile_pool(name="sbk", bufs=1) as sbk,
        tc.tile_pool(name="sbg", bufs=3) as sbg,
        tc.tile_pool(name="sst", bufs=G + 3) as sst,
        tc.tile_pool(name="sbu", bufs=2) as sbu,
        tc.tile_pool(name="sbo", bufs=3) as sbo,
        tc.tile_pool(name="p1", bufs=5, space="PSUM") as p1,   # conv chunks
        tc.tile_pool(name="p2", bufs=2, space="PSUM") as p2,   # guide
        tc.tile_pool(name="p3", bufs=1, space="PSUM") as p3,   # generator
    ):
        # ---- small weights on scalar ring; bulk on sync/scalar/pool ----
        wgp16 = sbw.tile([128, 5, 8], F16, tag="wgp16")
        nc.scalar.dma_start(wgp16[:], d_wgp16.ap())
        wl8, wh8 = [], []
        for m in range(3):
            tl = sbw.tile([128, 2, 16], FP8E5, tag=f"wl8_{m}")
            nc.scalar.dma_start(tl[:], d_wl.ap()[:, m, :, :])
            wl8.append(tl)
            th = sbw.tile([128, 2, 16], FP8, tag=f"wh8_{m}")
            nc.scalar.dma_start(th[:], d_wh.ap()[:, m, :, :])
            wh8.append(th)
        bg8 = sbw.tile([8, 1], F32, tag="bg8")
        nc.scalar.dma_start(bg8[:], d_bg.ap())

        krhs = sbk.tile([72, 4096], F16, tag="krhs")
        nc.sync.dma_start(krhs[:], d_krhs.ap())
        w1aug = sbw.tile([65, 64], F32, tag="w1aug")
        nc.sync.dma_start(w1aug[:], d_w1aug.ap())
        sdelta_t = sbw.tile([8, 72], F16, tag="sdelta")
        nc.sync.dma_start(sdelta_t[:], d_sdelta.ap())

        nb32 = sbw.tile([128, 1], F32, tag="nb32")
        nc.vector.memset(nb32[:], -BIG / 2)

        # x layouts, quarter-granular DMAs (fp16 on sync/scalar, fp8 on pool)
        xt = {}
        NQ = 8
        Q = (HW2 + NQ - 1) // NQ
        for n, dt in (("xa16", F16), ("x216", F16), ("xa8", FP8),
                      ("gxa8", FP8E5)):
            xt[n] = sbx.tile([128, HW2], dt, tag=n, name=n)
        for qi in range(NQ):
            lo, hi = Q * qi, min(Q * (qi + 1), HW2)
            nc.sync.dma_start(xt["xa16"][:, lo:hi], d_xs["xa16"].ap()[:, lo:hi])
            nc.scalar.dma_start(xt["x216"][:, lo:hi],
                                d_xs["x216"].ap()[:, lo:hi])
            for n in ("xa8", "gxa8"):
                nc.gpsimd.dma_start(xt[n][:, lo:hi], d_xs[n].ap()[:, lo:hi])
        xv16 = {n: xt[n][:].rearrange("p (h w) -> p h w", h=H)
                for n in ("xa16", "x216")}

        LW = sbw.tile([128, 16, 128], F16, tag="LW")
        LWE = sbw.tile([72, 4, 128], F16, tag="LWE")
        # mask-inject rows of LWE: BIG at (2c -> cols 0:64), (2c+1 -> 64:128)
        nc.scalar.dma_start(LWE[64:72, :, :], d_ec.ap())
        ones = sbw.tile([128, 64], F16, tag="ones")
        nc.sync.dma_start(ones[:], d_ones.ap())

        sts = {}

        def guide_tile(t):
            pg = p2.tile([40, TN], F32, tag="pg", name=f"pg{t}")
            for s, (xn, dy, dx, k) in enumerate(GMAIN):
                nc.tensor.matmul(pg[32:40, :], wgp16[0:k, s, :],
                                 _win(xv16[xn], t, dy, dx, k),
                                 start=(s == 0), stop=(s == 4),
                                 skip_group_check=True)
            # corr: pass A (wl*2^8 e5m2 @ xa8) + pass B (wh e4m3 @ gxa8)
            # mm0: ktiles dy0,dy1 -> taps 0,1,3,4
            # mm1: ktiles (dy2,dx0),(dy2,dx2) -> taps 6,7 + 8 (half zeroed)
            # mm2: ktiles (dy0,dx2),(dy1,dx2) -> taps 2,5 on top halves
            i = 0
            for wt, xn in ((wl8, "xa8"), (wh8, "gxa8")):
                for m, (ks, dy, dx) in enumerate(CORR_SPECS):
                    nc.tensor.matmul(pg[0:16, :], wt[m][:],
                                     _corr_ap(xt[xn], t, ks, dy, dx),
                                     start=(i == 0), stop=(i == 5),
                                     perf_mode=DR, skip_group_check=True)
                    i += 1
            gs = sbg.tile([8, TN], F32, tag="gs", name=f"gs{t}")
            nc.scalar.activation(gs[:], pg[32:40, :], AFT.Identity, bias=bg8[:])
            g = sbg.tile([8, TN], F32, tag="g", name=f"g{t}")
            nc.vector.scalar_tensor_tensor(g[:], pg[0:8, :], CSC, gs[:],
                                           op0=AL.mult, op1=AL.add)
            gm = sbg.tile([8, TN], F32, tag="gm", name=f"gm{t}")
            nc.gpsimd.partition_all_reduce(gm[:], g[:], channels=8,
                                           reduce_op=bass_isa.ReduceOp.max)
            st = sst.tile([72, TN], F16, tag="st", name=f"st{t}")
            stv = st[:].rearrange("p (r c) -> p r c", r=4)
            nc.scalar.copy(stv[0:64, :, :], _win(xv16["x216"], t, 2, 2, 64))
            nc.vector.tensor_tensor(st[64:72, :], g[:], gm[:], op=AL.is_equal)
            sts[t] = st

        # pooled rowsum reduce (DVE) overlaps the guide prologue on PE
        rs = sbk.tile([64, 378], F16, tag="rs")
        xrow = xt["xa16"][0:64, :].rearrange("p (a b) -> p a b", b=42)
        with nc.allow_low_precision(reason="pooled feeds sigmoid; fp16 ok"):
            for r0 in range(0, 378, 48):
                r1 = min(r0 + 48, 378)
                nc.vector.tensor_reduce(
                    rs[:, r0:r1], xrow[:, r0:r1, :],
                    axis=mybir.AxisListType.X, op=AL.add)
        # pooled cols in slot order: (0,1),(3,4),(6,7) at 0..5; 2,5,8 at 6..8
        pooled = sbg.tile([65, 9], F32, tag="pooled")
        rsv = rs[:].rearrange("p (kh r kw) -> p kh kw r", kh=3, r=42, kw=3)
        for kh in range(3):
            nc.vector.tensor_reduce(
                pooled[0:64, 2 * kh: 2 * kh + 2], rsv[:, kh, 0:2, :],
                axis=mybir.AxisListType.X, op=AL.add)
            nc.vector.tensor_reduce(
                pooled[0:64, 6 + kh: 7 + kh], rsv[:, kh, 2:3, :],
                axis=mybir.AxisListType.X, op=AL.add)
        nc.vector.memset(pooled[64:65, :], 1.0)

        for t in range(GPRE):
            guide_tile(t)

        # ---- generator: s1 -> kern (fp16) -> LW/LWE via pool-ring DMAs ----
        s1p = p3.tile([64, 9], F32, tag="pk", name="s1p")
        nc.tensor.matmul(s1p[:], w1aug[:], pooled[:], start=True, stop=True)
        s1s = sbg.tile([64, 9], F16, tag="s1s")
        nc.scalar.activation(s1s[:], s1p[:], AFT.Sigmoid)

        S = sbk.tile([72, 72], F16, tag="S")
        nc.vector.memset(S[:], 0.0)
        nc.scalar.copy(S[64:72, :], sdelta_t[:])
        for gi in range(8):
            nc.sync.dma_start(S[8 * gi:8 * gi + 8, 9 * gi:9 * gi + 9],
                              s1s[8 * gi:8 * gi + 8, :])

        ksb = sbk.tile([72, 4096], F16, tag="ksb")
        for j in range(8):
            pk = p3.tile([72, 512], F32, tag="pk", name=f"pk{j}")
            nc.tensor.matmul(pk[:], S[:], krhs[:, 512 * j: 512 * (j + 1)],
                             start=True, stop=True)
            nc.scalar.copy(ksb[:, 512 * j: 512 * (j + 1)], pk[:])

        # LW[s*64+ci, c*4+p, h*64+co] <- ksb[g*9+2p+s, ci*64+co], g = 2c+h
        ksv = ksb[:].rearrange("p (ci co) -> p ci co", ci=64)
        for c in range(4):
            for p in range(4):
                for hh in range(2):
                    gg = 2 * c + hh
                    nc.gpsimd.dma_start(
                        LW[0:128, c * 4 + p, 64 * hh:64 * hh + 64],
                        ksv[gg * 9 + 2 * p: gg * 9 + 2 * p + 2, :, :])
            for hh in range(2):
                gg = 2 * c + hh
                nc.gpsimd.dma_start(
                    LWE[0:64, c, 64 * hh:64 * hh + 64],
                    ksv[gg * 9 + 8: gg * 9 + 9, :, :])

        for t in range(GPRE, G):
            guide_tile(t)

        # ---- main loop: guide(t+G) + conv(t), relu-select, add-tree ----
        def conv_tile(t):
            st = sts.pop(t)
            pcs = []
            for c in range(4):
                pc = p1.tile([128, TN], F32, tag="pc", name=f"pc{c}_{t}")
                for p, (xn, dy, dx) in enumerate(CPAIRS):
                    nc.tensor.matmul(pc[:], LW[0:128, 4 * c + p, :],
                                     _win(xv16[xn], t, dy, dx, 128),
                                     start=(p == 0), stop=False)
                nc.tensor.matmul(pc[:], LWE[0:72, c, :], st[:],
                                 start=False, stop=True)
                pcs.append(pc)
            us = []
            for c in range(4):
                u = sbu.tile([128, TN], F16, tag=f"u{c}", name=f"u{c}_{t}")
                if c < 3:
                    nc.scalar.activation(u[:], pcs[c][:], AFT.Relu,
                                         bias=nb32[:])
                else:
                    nc.vector.tensor_scalar(u[:], pcs[c][:], -BIG / 2, 0.0,
                                            op0=AL.add, op1=AL.max)
                us.append(u)
            v0 = sbu.tile([128, TN], F16, tag="v0", name=f"v0_{t}")
            nc.vector.tensor_tensor(v0[:], us[0][:], us[1][:], op=AL.add)
            v1 = sbu.tile([128, TN], F16, tag="v1", name=f"v1_{t}")
            nc.vector.tensor_tensor(v1[:], us[2][:], us[3][:], op=AL.add)
            wv = sbu.tile([128, TN], F16, tag="wv", name=f"wv_{t}")
            nc.vector.tensor_tensor(wv[:], v0[:], v1[:], op=AL.add)
            pf = p3.tile([64, TN], F32, tag="pk", name=f"pf{t}")
            nc.tensor.matmul(pf[:], ones[:], wv[:], start=True, stop=True)
            ot = sbo.tile([64, TN], F32, tag="ot", name=f"ot{t}")
            nc.scalar.activation(ot[:], pf[:], AFT.Copy, bias=-BIG / 2)
            nc.sync.dma_start(d_y.ap()[:, TN * t: TN * (t + 1)], ot[:])

        for t in range(NT):
            if t + G < NT:
                guide_tile(t + G)
            conv_tile(t)


def host_prep(w1, b1, w2, b2, wg, bg):
    w1p = (np.asarray(w1, np.float32).T / 1764.0)
    w1aug = np.concatenate([w1p, np.asarray(b1, np.float32)[None, :]], 0)

    # kern rows (g, slot): w2r/b2r ordered so slot s corresponds to hw
    # SLOT_HW[s]; w2 columns (i) follow s1's hw->slot ordering via pooled.
    w2v = np.asarray(w2, np.float32).reshape(R, COUT, CIN, R)
    w2r = w2v.transpose(0, 3, 2, 1).reshape(64, CIN * COUT)
    b2v = np.asarray(b2, np.float32).reshape(R, COUT, CIN)
    b2r = b2v.transpose(0, 2, 1).reshape(R, CIN * COUT)
    krhs = np.ascontiguousarray(
        np.concatenate([w2r, b2r], 0).astype(np.float16))

    sdelta = np.zeros((8, 72), np.float16)
    for g in range(8):
        sdelta[g, 9 * g: 9 * g + 9] = 1.0

    wgv = np.asarray(wg, np.float32)          # [8, 64, 3, 3]
    wg16 = wgv.astype(np.float16)
    wgl = wgv - wg16.astype(np.float32)       # fp16 residual

    def tapw(a, hw):
        dy, dx = hw // 3, hw % 3
        return a[:, :, dy, dx].T              # [cin, 8]

    # guide main fp16: 5 slots (pairs + tap8 single)
    wgp = np.zeros((128, 5, 8), np.float16)
    for s in range(4):
        wgp[0:64, s, :] = tapw(wg16, SLOT_HW[2 * s])
        wgp[64:128, s, :] = tapw(wg16, SLOT_HW[2 * s + 1])
    wgp[0:64, 4, :] = tapw(wg16, 8)

    # guide corr DR lhsT [128, mm, ktile, 8]:
    # mm0 ktiles (dy0, dy1): taps (0,1), (3,4)
    # mm1 ktiles (dy2 dx0, dy2 dx2): taps (6,7), (8, zero)
    # mm2 ktiles (dy0 dx2, dy1 dx2): taps (2, zero), (5, zero)
    def corr_pack(a, dtype):
        p = np.zeros((128, 3, 2, 16), np.float32)
        p[0:64, 0, 0, 0:8] = tapw(a, 0)
        p[64:128, 0, 0, 0:8] = tapw(a, 1)
        p[0:64, 0, 1, 0:8] = tapw(a, 3)
        p[64:128, 0, 1, 0:8] = tapw(a, 4)
        p[0:64, 1, 0, 0:8] = tapw(a, 6)
        p[64:128, 1, 0, 0:8] = tapw(a, 7)
        p[0:64, 1, 1, 0:8] = tapw(a, 8)
        p[0:64, 2, 0, 0:8] = tapw(a, 2)
        p[0:64, 2, 1, 0:8] = tapw(a, 5)
        return p.astype(dtype)

    wl8 = corr_pack(wgl * 256.0, ml_dtypes.float8_e5m2)
    wh8 = corr_pack(wg16.astype(np.float32), ml_dtypes.float8_e4m3)

    bg8 = np.asarray(bg, np.float32).reshape(8, 1)

    ec16 = np.zeros((8, 4, 128), np.float16)
    for c in range(4):
        ec16[2 * c, c, 0:64] = BIG
        ec16[2 * c + 1, c, 64:128] = BIG

    ones64 = np.zeros((128, 64), np.float16)
    for k in range(128):
        ones64[k, k % 64] = 1.0

    return dict(w1aug=np.ascontiguousarray(w1aug), krhs=krhs, sdelta=sdelta,
                wgp16=wgp, wl8=wl8, wh8=wh8, bg8=bg8, ec16=ec16,
                ones64=ones64)


def shard_x(xb):
    """One sample [64, 126, 126] -> 4 shifted SBUF layouts."""
    xf = np.ascontiguousarray(np.asarray(xb, np.float32).reshape(CIN, HW2))
    x16 = xf.astype(np.float16)
    x8 = xf.astype(ml_dtypes.float8_e4m3)
    gl = ((xf - x16.astype(np.float32)) * 256.0).astype(ml_dtypes.float8_e5m2)

    def stack(a, shift, dtype):
        t = np.zeros((128, HW2), dtype)
        t[0:64] = a
        t[64:128, 0:HW2 - shift] = a[:, shift:]
        return t

    return {
        "xa16": stack(x16, 1, np.float16),
        "x216": stack(x16, 126, np.float16),
        "xa8": stack(x8, 1, ml_dtypes.float8_e4m3),
        "gxa8": stack(gl, 1, ml_dtypes.float8_e5m2),
    }


_NC_CACHE = {}


def kernel(x, w1, b1, w2, b2, wg, bg, _profile=None):
    x = np.asarray(x, np.float32)
    Bn = x.shape[0]
    assert Bn == 8
    weights = host_prep(w1, b1, w2, b2, wg, bg)

    if "nc" not in _NC_CACHE:
        _NC_CACHE["nc"] = build_module()
    nc = _NC_CACHE["nc"]

    in_maps = []
    for b in range(Bn):
        m = shard_x(x[b])
        m.update(weights)
        in_maps.append(m)

    kwargs = dict(_profile.get("kwargs", {})) if _profile else {}
    res = run_bass_kernel_spmd(nc, in_maps, core_ids=list(range(Bn)), **kwargs)
    if _profile is not None:
        _profile["result"] = res

    out = np.stack([res.results[b]["y"].reshape(COUT, HO, WO)
                    for b in range(Bn)])
    return out.astype(np.float32)
